# revision 24
# baseline (speedup 1.0000x reference)
"""NGCF (3-layer graph message passing) on 8 TRN2 NeuronCores.

Sharding: nodes (rows of the [100000,64] embedding table and of the
segment_sum output) split into 8 slices of 12500 rows; edges partitioned
by destination core.  Per layer: each core gathers source rows from a
replicated (all-gathered) full table, scales by edge value, and
segment-sums into its own 12500-row `side` slice held in SBUF via
one-hot matmuls on the PE array (HW dma_scatter_add drops duplicate
indices, so scatter is avoided entirely).  The dense
side@W_gc + (ego*side)@W_bi + b -> leaky_relu runs locally per 128-row
window, then the new layer embeddings are all-gathered.  Final read-out
(batch gather, l2-normalize, concat, dot) also runs on-device.
"""

import os
import sys

import numpy as np

if "/opt/trn_rl_repo" not in sys.path:
    sys.path.insert(0, "/opt/trn_rl_repo")

import concourse.bacc as bacc
import concourse.mybir as mybir
from concourse.bass_utils import run_bass_kernel_spmd
from concourse.masks import make_identity
from concourse.tile import TileContext

N_USER = 50000
N_ITEM = 50000
N = N_USER + N_ITEM
D = 64
L = 3
NC = 8
RPC = N // NC            # 12500 rows per core
NT = 98                  # ceil(12500/128) windows
RPC_PAD = NT * 128       # 12544
CHUNK = 25000            # gather chunk size (int16-safe local indices)
NCH = N // CHUNK         # 4
EB = 512                 # edges per gather block (desc-ring safe size)
BPG = EB // 128          # 32 matmul blocks per gather
BATCH = 4096
BPC = BATCH // NC        # 512 batch entries per core
CAP2 = 256               # slots per (u-chunk, i-chunk) pair group
PADB = 4 * CAP2          # 1024 output rows per core
EPS = 1e-12

TRACE_ENV = "NGCF_KERNEL_TRACE"
LAST = {}

f32 = mybir.dt.float32
i16 = mybir.dt.int16


def _build_program(NG_j, blkmap):
    NGT = sum(NG_j)
    nc = bacc.Bacc(None, target_bir_lowering=False)

    ego0_full = nc.declare_dram_parameter("ego0_full", [N, D], f32, isOutput=False)
    ego0_own = nc.declare_dram_parameter("ego0_own", [RPC_PAD, D], f32, isOutput=False)
    w_cat = nc.declare_dram_parameter("w_cat", [L, 2 * D, D], f32, isOutput=False)
    b_rep = nc.declare_dram_parameter("b_rep", [L, 128, D], f32, isOutput=False)
    iota_m = nc.declare_dram_parameter("iota_m", [128, 128], f32, isOutput=False)
    gidx = nc.declare_dram_parameter("gidx", [NGT, 128, EB // 16], i16, isOutput=False)
    vld = nc.declare_dram_parameter("vld", [NGT, 128, 2, BPG], f32, isOutput=False)
    fidx = nc.declare_dram_parameter("fidx", [2, 4, 128, CAP2 // 16], i16, isOutput=False)
    u_part = nc.declare_dram_parameter("u_part", [PADB, 4 * D], f32, isOutput=True)
    i_part = nc.declare_dram_parameter("i_part", [PADB, 4 * D], f32, isOutput=True)
    pos_part = nc.declare_dram_parameter("pos_part", [PADB], f32, isOutput=True)

    mines = [nc.dram_tensor(f"mine{k}", [RPC, D], f32, kind="Internal") for k in range(L)]
    egofs = [
        nc.dram_tensor(f"egof{k}", [N, D], f32, kind="Internal", addr_space="Shared")
        for k in range(L)
    ]

    mult = mybir.AluOpType.mult
    add = mybir.AluOpType.add
    amax = mybir.AluOpType.max
    iseq = mybir.AluOpType.is_equal

    with TileContext(nc) as tc:
        with (
            tc.tile_pool(name="const", bufs=1) as cpool,
            tc.tile_pool(name="meta", bufs=4) as mpool,
            tc.tile_pool(name="pay", bufs=3) as ppool,
            tc.tile_pool(name="dense", bufs=3) as dpool,
            tc.tile_pool(name="pse", bufs=4, space="PSUM") as pepool,
            tc.tile_pool(name="psd", bufs=2, space="PSUM") as pspool,
            tc.tile_pool(name="fin", bufs=2) as fpool,
        ):
            wtile = cpool.tile([2 * D, L * D], f32)
            btile = cpool.tile([128, L * D], f32)
            for k in range(L):
                nc.sync.dma_start(wtile[:, k * D:(k + 1) * D], w_cat[k, :, :])
                nc.sync.dma_start(btile[:, k * D:(k + 1) * D], b_rep[k, :, :])
            ident = cpool.tile([128, 128], f32)
            make_identity(nc, ident[:])
            iota = cpool.tile([128, 128], f32)
            nc.sync.dma_start(iota[:], iota_m[:, :])
            nreg_eb = nc.gpsimd.to_reg(EB)
            nreg_cap = nc.gpsimd.to_reg(CAP2)

            side_s = cpool.tile([128, NT, D], f32)
            eown_s = cpool.tile([128, NT, D], f32)
            nc.sync.dma_start(eown_s[:], ego0_own[:, :].rearrange("(f p) d -> p f d", f=NT))

            for k in range(L):
                src_full = ego0_full if k == 0 else egofs[k - 1]
                nc.vector.memset(side_s[:], 0.0)

                gpos = 0
                for j in range(NCH):
                    src_ap = src_full[j * CHUNK:(j + 1) * CHUNK, :]
                    for g in range(NG_j[j]):
                        gi = mpool.tile([128, EB // 16], i16)
                        vl = mpool.tile([128, 2, BPG], f32)
                        nc.sync.dma_start(gi[:], gidx[gpos, :, :])
                        nc.sync.dma_start(vl[:], vld[gpos, :, :, :])
                        xg = ppool.tile([128, BPG, D], f32)
                        nc.gpsimd.dma_gather(xg[:], src_ap, gi[:], EB, nreg_eb, D)
                        vb = vl[:, 0, :, None].broadcast_to([128, BPG, D])
                        nc.vector.tensor_tensor(out=xg[:], in0=xg[:], in1=vb, op=mult)
                        oh = ppool.tile([128, BPG, 128], f32)
                        lb = vl[:, 1, :, None].broadcast_to([128, BPG, 128])
                        ib = iota[:, None, :].broadcast_to([128, BPG, 128])
                        nc.vector.tensor_tensor(out=oh[:], in0=lb, in1=ib, op=iseq)
                        for b in range(BPG):
                            w, first, last = blkmap[j][g * BPG + b]
                            if first:
                                pst = pepool.tile([128, D], f32)
                            nc.tensor.matmul(
                                pst[:, :], lhsT=oh[:, b, :], rhs=xg[:, b, :],
                                start=first, stop=last,
                            )
                            if last:
                                nc.vector.tensor_tensor(
                                    out=side_s[:, w, :], in0=side_s[:, w, :],
                                    in1=pst[:, :], op=add,
                                )
                        gpos += 1

                for t in range(NT):
                    r0 = t * 128
                    P = min(128, RPC - r0)
                    sz = dpool.tile([128, 2 * D], f32)
                    nc.scalar.copy(sz[:, 0:D], side_s[:, t, :])
                    nc.vector.tensor_mul(sz[:, D:2 * D], side_s[:, t, :], eown_s[:, t, :])
                    szT_p = pspool.tile([128, 128], f32)
                    nc.tensor.transpose(out=szT_p[:], in_=sz[:], identity=ident[:])
                    szT_s = dpool.tile([128, 128], f32)
                    nc.scalar.copy(szT_s[:], szT_p[:])
                    y_p = pspool.tile([128, D], f32)
                    nc.tensor.matmul(
                        y_p[:], lhsT=szT_s[:], rhs=wtile[:, k * D:(k + 1) * D],
                        start=True, stop=True,
                    )
                    yt = dpool.tile([128, D], f32)
                    nc.vector.tensor_add(yt[:], y_p[:], btile[:, k * D:(k + 1) * D])
                    nc.vector.scalar_tensor_tensor(
                        out=yt[:], in0=yt[:], scalar=0.2, in1=yt[:],
                        op0=mult, op1=amax,
                    )
                    nc.scalar.copy(eown_s[:, t, :], yt[:])
                    nc.sync.dma_start(mines[k][r0:r0 + P, :], yt[:P, :])

                nc.gpsimd.collective_compute(
                    "AllGather",
                    mybir.AluOpType.bypass,
                    replica_groups=[list(range(NC))],
                    ins=[mines[k][:, :]],
                    outs=[egofs[k][:, :]],
                )

            srcs = [ego0_full] + egofs
            accs = []
            for s_i in range(2):
                acc = fpool.tile([128, 8, 4 * D], f32, name=f"acc{s_i}")
                for ti in range(4):
                    src = srcs[ti]
                    for p in range(4):
                        jch = (p // 2) if s_i == 0 else (2 + p % 2)
                        fi = fpool.tile([128, CAP2 // 16], i16)
                        nc.sync.dma_start(fi[:], fidx[s_i, p, :, :])
                        g = fpool.tile([128, 2, D], f32)
                        nc.gpsimd.dma_gather(
                            g[:], src[jch * CHUNK:(jch + 1) * CHUNK, :], fi[:], CAP2, nreg_cap, D
                        )
                        dst = acc[:, 2 * p:2 * p + 2, ti * D:(ti + 1) * D]
                        if ti == 0:
                            nc.scalar.copy(dst, g[:])
                        else:
                            sq = fpool.tile([128, 2, D], f32)
                            nc.vector.tensor_mul(sq[:], g[:], g[:])
                            nr = fpool.tile([128, 2], f32)
                            nc.vector.reduce_sum(nr[:], sq[:], axis=mybir.AxisListType.X)
                            nc.scalar.activation(nr[:], nr[:], mybir.ActivationFunctionType.Sqrt)
                            nc.vector.tensor_scalar_max(nr[:], nr[:], EPS)
                            nc.vector.reciprocal(nr[:], nr[:])
                            nb = nr[:, :, None].broadcast_to([128, 2, D])
                            nc.vector.tensor_tensor(out=dst, in0=g[:], in1=nb, op=mult)
                accs.append(acc)
            u_acc, i_acc = accs
            prod = fpool.tile([128, 8, 4 * D], f32)
            nc.vector.tensor_mul(prod[:], u_acc[:], i_acc[:])
            pos_t = fpool.tile([128, 8], f32)
            nc.vector.reduce_sum(pos_t[:], prod[:], axis=mybir.AxisListType.X)
            nc.sync.dma_start(u_part[:, :].rearrange("(p f) d -> p f d", f=8), u_acc[:])
            nc.sync.dma_start(i_part[:, :].rearrange("(p f) d -> p f d", f=8), i_acc[:])
            nc.sync.dma_start(pos_part[:].rearrange("(p f) -> p f", f=8), pos_t[:])

    nc.finalize()
    return nc


def _run_timed(nc, in_maps, iters):
    # mirror of bass2jax.run_bass_via_pjrt multi-core branch, but keeps the
    # jitted executable so repeated launches can be wall-clock timed.
    import time

    import jax
    from jax.experimental.shard_map import shard_map
    from jax.sharding import Mesh, NamedSharding, PartitionSpec

    from concourse import bass2jax

    bass2jax.install_neuronx_cc_hook()
    partition_name = nc.partition_id_tensor.name if nc.partition_id_tensor else None
    in_names, out_names, out_avals, zero_outs = [], [], [], []
    for alloc in nc.m.functions[0].allocations:
        if not isinstance(alloc, mybir.MemoryLocationSet):
            continue
        name = alloc.memorylocations[0].name
        if alloc.kind == "ExternalInput":
            if name != partition_name:
                in_names.append(name)
        elif alloc.kind == "ExternalOutput":
            shape = tuple(alloc.tensor_shape)
            dtype = mybir.dt.np(alloc.dtype)
            out_names.append(name)
            out_avals.append(jax.core.ShapedArray(shape, dtype))
            zero_outs.append(np.zeros(shape, dtype))
    n_params = len(in_names)
    n_outs = len(out_avals)
    in_names.extend(out_names)
    if partition_name is not None:
        in_names.append(partition_name)
    donate = tuple(range(n_params, n_params + n_outs))

    def _body(*args):
        operands = list(args)
        if partition_name is not None:
            operands.append(bass2jax.partition_id_tensor())
        outs = bass2jax._bass_exec_p.bind(
            *operands,
            out_avals=tuple(out_avals),
            in_names=tuple(in_names),
            out_names=tuple(out_names),
            lowering_input_output_aliases=(),
            sim_require_finite=True,
            sim_require_nnan=True,
            nc=nc,
        )
        return tuple(outs)

    devices = jax.devices()[:NC]
    mesh = Mesh(np.asarray(devices), ("core",))
    in_specs = (PartitionSpec("core"),) * (n_params + n_outs)
    out_specs = (PartitionSpec("core"),) * len(out_names)
    sharded = jax.jit(
        shard_map(_body, mesh=mesh, in_specs=in_specs, out_specs=out_specs, check_rep=False),
        donate_argnums=donate,
        keep_unused=True,
    )
    per_core = [[np.asarray(m[name]) for name in in_names[:n_params]] for m in in_maps]
    concat_in = [
        np.concatenate([per_core[c][i] for c in range(NC)], axis=0)
        for i in range(n_params)
    ]
    concat_zeros = [
        np.zeros((NC * z.shape[0], *z.shape[1:]), z.dtype) for z in zero_outs
    ]
    sh = NamedSharding(mesh, PartitionSpec("core"))
    dev_in = [jax.device_put(a, sh) for a in concat_in]
    out_arrs = sharded(*dev_in, *[jax.device_put(z, sh) for z in concat_zeros])
    jax.block_until_ready(out_arrs)
    results = [
        {
            name: np.asarray(out_arrs[i]).reshape(NC, *out_avals[i].shape)[c]
            for i, name in enumerate(out_names)
        }
        for c in range(NC)
    ]
    times = []
    for _ in range(iters):
        zs = [jax.device_put(z, sh) for z in concat_zeros]
        jax.block_until_ready(zs)
        t0 = time.perf_counter()
        o = sharded(*dev_in, *zs)
        jax.block_until_ready(o)
        times.append(time.perf_counter() - t0)
    LAST["times"] = times
    return results, (min(times) if times else None)


def _wrap_idx(arr2d):
    # [nb, n] -> [nb, 128, n//16]; idx layout wrapped[p, s] = x[s*16 + p%16]
    nb, n = arr2d.shape
    w = arr2d.reshape(nb, n // 16, 16).transpose(0, 2, 1)
    return np.ascontiguousarray(np.tile(w, (1, 8, 1)))


def kernel(user, item_i, adj_row, adj_col, adj_val, user_emb, item_emb, W_gc, b_gc, W_bi, b_bi):
    user = np.asarray(user)
    item_i = np.asarray(item_i)
    adj_row = np.asarray(adj_row).astype(np.int64)
    adj_col = np.asarray(adj_col).astype(np.int64)
    adj_val = np.asarray(adj_val, dtype=np.float32)
    user_emb = np.asarray(user_emb, dtype=np.float32)
    item_emb = np.asarray(item_emb, dtype=np.float32)
    W_gc = np.asarray(W_gc, dtype=np.float32)
    b_gc = np.asarray(b_gc, dtype=np.float32)
    W_bi = np.asarray(W_bi, dtype=np.float32)
    b_bi = np.asarray(b_bi, dtype=np.float32)

    ego0 = np.ascontiguousarray(np.concatenate([user_emb, item_emb], axis=0))
    w_cat = np.ascontiguousarray(np.concatenate([W_gc, W_bi], axis=1))  # [L,128,64]
    b_sum = (b_gc[:, 0, :] + b_bi[:, 0, :])[:, None, :]                 # [L,1,64]
    b_rep = np.ascontiguousarray(np.broadcast_to(b_sum, (L, 128, D)), dtype=np.float32)
    iota_m = np.broadcast_to(np.arange(128, dtype=np.float32)[None, :], (128, 128))
    iota_m = np.ascontiguousarray(iota_m)

    # ---- edge partition: sort by (dest core, source chunk, dest window) ----
    core = adj_row // RPC
    r_loc = adj_row % RPC
    w_arr = r_loc // 128
    ld_arr = (r_loc % 128).astype(np.float32)
    chunk = adj_col // CHUNK
    col_loc = (adj_col % CHUNK).astype(np.int16)
    key = (core * NCH + chunk) * NT + w_arr
    order = np.argsort(key, kind="stable")
    col_s = col_loc[order]
    val_s = adj_val[order]
    ld_s = ld_arr[order]
    bounds = np.searchsorted(key[order], np.arange(NC * NCH * NT + 1))
    cnt = np.diff(bounds).reshape(NC, NCH, NT)

    # blocks per (chunk, window): max over cores, so one program fits all
    NBW = (-(-cnt // 128)).max(axis=0)          # [NCH, NT]
    pad_j = (-NBW.sum(axis=1)) % BPG
    NBW[:, NT - 1] += pad_j                     # pad to whole gathers on last window
    NG_j = [int(NBW[j].sum()) // BPG for j in range(NCH)]
    NGT = sum(NG_j)
    offs = np.zeros((NCH, NT), np.int64)        # block offset of window w in chunk j
    for j in range(NCH):
        offs[j, 1:] = np.cumsum(NBW[j])[:-1]
    blkmap = []
    for j in range(NCH):
        m = []
        for w in range(NT):
            nb = int(NBW[j, w])
            for b in range(nb):
                m.append((w, b == 0, b == nb - 1))
        blkmap.append(m)

    gidx_all = np.empty((NC, NGT, 128, EB // 16), np.int16)
    vld_all = np.empty((NC, NGT, 128, 2, BPG), np.float32)
    for c in range(NC):
        gpos = 0
        for j in range(NCH):
            totb = int(NBW[j].sum())
            cl = np.zeros(totb * 128, np.int16)
            vv = np.zeros(totb * 128, np.float32)
            lv = np.zeros(totb * 128, np.float32)
            for w in range(NT):
                kk = (c * NCH + j) * NT + w
                s, e = int(bounds[kk]), int(bounds[kk + 1])
                o = int(offs[j, w]) * 128
                cl[o:o + e - s] = col_s[s:e]
                vv[o:o + e - s] = val_s[s:e]
                lv[o:o + e - s] = ld_s[s:e]
            ng = totb // BPG
            gidx_all[c, gpos:gpos + ng] = _wrap_idx(cl.reshape(ng, EB))
            vld_all[c, gpos:gpos + ng, :, 0, :] = vv.reshape(ng, BPG, 128).transpose(0, 2, 1)
            vld_all[c, gpos:gpos + ng, :, 1, :] = lv.reshape(ng, BPG, 128).transpose(0, 2, 1)
            gpos += ng

    # ---- final-phase batch grouping by (u-chunk, i-chunk) pair ----
    fidx_all = np.empty((NC, 2, 4, 128, CAP2 // 16), np.int16)
    slots_all = []
    for c in range(NC):
        u_ids = user[c * BPC:(c + 1) * BPC].astype(np.int64)
        i_ids = item_i[c * BPC:(c + 1) * BPC].astype(np.int64) + N_USER
        pair = (u_ids // CHUNK) * 2 + (i_ids // CHUNK - 2)
        slot_of = np.empty(BPC, np.int64)
        fx = np.zeros((2, 4, CAP2), np.int64)
        for p in range(4):
            pos = np.nonzero(pair == p)[0]
            pcnt = len(pos)
            assert pcnt <= CAP2, f"pair group overflow: core {c} pair {p} cnt {pcnt}"
            rank = np.arange(pcnt)
            slot_of[pos] = (rank % 128) * 8 + 2 * p + rank // 128
            fx[0, p, :pcnt] = u_ids[pos] - (p // 2) * CHUNK
            fx[1, p, :pcnt] = i_ids[pos] - (2 + p % 2) * CHUNK
        fw = fx.astype(np.int16).reshape(2, 4, CAP2 // 16, 16).transpose(0, 1, 3, 2)
        fidx_all[c] = np.tile(fw, (1, 1, 8, 1))
        slots_all.append(slot_of)

    nc = _build_program(NG_j, blkmap)

    ego0_pad = np.zeros((RPC_PAD, D), np.float32)
    in_maps = []
    for c in range(NC):
        ego0_pad_c = ego0_pad.copy()
        ego0_pad_c[:RPC] = ego0[c * RPC:(c + 1) * RPC]
        in_maps.append({
            "ego0_full": ego0,
            "ego0_own": ego0_pad_c,
            "w_cat": w_cat,
            "b_rep": b_rep,
            "iota_m": iota_m,
            "gidx": gidx_all[c],
            "vld": vld_all[c],
            "fidx": np.ascontiguousarray(fidx_all[c]),
        })

    trace = os.environ.get(TRACE_ENV, "0") == "1"
    time_iters = int(os.environ.get("NGCF_TIME_ITERS", "0"))
    if time_iters > 0 and not trace:
        res_list, t_best = _run_timed(nc, in_maps, time_iters)
        LAST["exec_time_ns"] = None if t_best is None else int(t_best * 1e9)
    else:
        res = run_bass_kernel_spmd(nc, in_maps, core_ids=list(range(NC)), trace=trace)
        LAST["exec_time_ns"] = res.exec_time_ns
        res_list = res.results

    u_g = np.empty((BATCH, 4 * D), np.float32)
    i_g = np.empty((BATCH, 4 * D), np.float32)
    pos = np.empty((BATCH,), np.float32)
    for c in range(NC):
        so = slots_all[c]
        rc = res_list[c]
        u_g[c * BPC:(c + 1) * BPC] = rc["u_part"][so]
        i_g[c * BPC:(c + 1) * BPC] = rc["i_part"][so]
        pos[c * BPC:(c + 1) * BPC] = rc["pos_part"][so]
    return (u_g, i_g, pos)


# revision 27
# speedup vs baseline: 1.0203x; 1.0203x over previous
"""NGCF (3-layer graph message passing) on 8 TRN2 NeuronCores.

Sharding: nodes (rows of the [100000,64] embedding table and of the
segment_sum output) split into 8 slices of 12500 rows; edges partitioned
by destination core.  Per layer: each core gathers source rows from a
replicated (all-gathered) full table, scales by edge value, and
segment-sums into its own 12500-row `side` slice held in SBUF via
one-hot matmuls on the PE array (HW dma_scatter_add drops duplicate
indices, so scatter is avoided entirely).  The dense
side@W_gc + (ego*side)@W_bi + b -> leaky_relu runs locally per 128-row
window, then the new layer embeddings are all-gathered.  Final read-out
(batch gather, l2-normalize, concat, dot) also runs on-device.
"""

import os
import sys

import numpy as np

if "/opt/trn_rl_repo" not in sys.path:
    sys.path.insert(0, "/opt/trn_rl_repo")

import concourse.bacc as bacc
import concourse.mybir as mybir
from concourse.bass_utils import run_bass_kernel_spmd
from concourse.masks import make_identity
from concourse.tile import TileContext

N_USER = 50000
N_ITEM = 50000
N = N_USER + N_ITEM
D = 64
L = 3
NC = 8
RPC = N // NC            # 12500 rows per core
NT = 98                  # ceil(12500/128) windows
RPC_PAD = NT * 128       # 12544
CHUNK = 25000            # gather chunk size (int16-safe local indices)
NCH = N // CHUNK         # 4
EB = 512                 # edges per gather block (desc-ring safe size)
BPG = EB // 128          # 32 matmul blocks per gather
BATCH = 4096
BPC = BATCH // NC        # 512 batch entries per core
CAP2 = 256               # slots per (u-chunk, i-chunk) pair group
PADB = 4 * CAP2          # 1024 output rows per core
EPS = 1e-12

TRACE_ENV = "NGCF_KERNEL_TRACE"
LAST = {}

f32 = mybir.dt.float32
bf16 = mybir.dt.bfloat16
i16 = mybir.dt.int16


def _build_program(NG_j, blkmap):
    NGT = sum(NG_j)
    nc = bacc.Bacc(None, target_bir_lowering=False)

    ego0_full = nc.declare_dram_parameter("ego0_full", [N, D], f32, isOutput=False)
    ego0_own = nc.declare_dram_parameter("ego0_own", [RPC_PAD, D], f32, isOutput=False)
    w_cat = nc.declare_dram_parameter("w_cat", [L, 2 * D, D], f32, isOutput=False)
    b_rep = nc.declare_dram_parameter("b_rep", [L, 128, D], f32, isOutput=False)
    iota_m = nc.declare_dram_parameter("iota_m", [128, 128], f32, isOutput=False)
    gidx = nc.declare_dram_parameter("gidx", [NGT, 128, EB // 16], i16, isOutput=False)
    vld = nc.declare_dram_parameter("vld", [NGT, 128, 2, BPG], f32, isOutput=False)
    fidx = nc.declare_dram_parameter("fidx", [2, 4, 128, CAP2 // 16], i16, isOutput=False)
    u_part = nc.declare_dram_parameter("u_part", [PADB, 4 * D], f32, isOutput=True)
    i_part = nc.declare_dram_parameter("i_part", [PADB, 4 * D], f32, isOutput=True)
    pos_part = nc.declare_dram_parameter("pos_part", [PADB], f32, isOutput=True)

    mines = [nc.dram_tensor(f"mine{k}", [RPC, D], f32, kind="Internal") for k in range(L)]
    egofs = [
        nc.dram_tensor(f"egof{k}", [N, D], f32, kind="Internal", addr_space="Shared")
        for k in range(L)
    ]

    mult = mybir.AluOpType.mult
    add = mybir.AluOpType.add
    amax = mybir.AluOpType.max
    iseq = mybir.AluOpType.is_equal

    with TileContext(nc) as tc:
        with (
            tc.tile_pool(name="const", bufs=1) as cpool,
            tc.tile_pool(name="meta", bufs=4) as mpool,
            tc.tile_pool(name="pay", bufs=3) as ppool,
            tc.tile_pool(name="dense", bufs=3) as dpool,
            tc.tile_pool(name="pse", bufs=4, space="PSUM") as pepool,
            tc.tile_pool(name="psd", bufs=2, space="PSUM") as pspool,
            tc.tile_pool(name="fin", bufs=2) as fpool,
        ):
            wtile = cpool.tile([2 * D, L * D], f32)
            btile = cpool.tile([128, L * D], f32)
            for k in range(L):
                nc.sync.dma_start(wtile[:, k * D:(k + 1) * D], w_cat[k, :, :])
                nc.sync.dma_start(btile[:, k * D:(k + 1) * D], b_rep[k, :, :])
            ident = cpool.tile([128, 128], f32)
            make_identity(nc, ident[:])
            iota = cpool.tile([128, 128], f32)
            nc.sync.dma_start(iota[:], iota_m[:, :])
            nreg_eb = nc.gpsimd.to_reg(EB)
            nreg_cap = nc.gpsimd.to_reg(CAP2)

            side_s = cpool.tile([128, NT, D], f32)
            eown_s = cpool.tile([128, NT, D], f32)
            nc.sync.dma_start(eown_s[:], ego0_own[:, :].rearrange("(f p) d -> p f d", f=NT))

            for k in range(L):
                src_full = ego0_full if k == 0 else egofs[k - 1]
                nc.vector.memset(side_s[:], 0.0)

                gpos = 0
                for j in range(NCH):
                    src_ap = src_full[j * CHUNK:(j + 1) * CHUNK, :]
                    for g in range(NG_j[j]):
                        gi = mpool.tile([128, EB // 16], i16)
                        vl = mpool.tile([128, 2, BPG], f32)
                        nc.sync.dma_start(gi[:], gidx[gpos, :, :])
                        nc.sync.dma_start(vl[:], vld[gpos, :, :, :])
                        xg = ppool.tile([128, BPG, D], f32)
                        nc.gpsimd.dma_gather(xg[:], src_ap, gi[:], EB, nreg_eb, D)
                        xgb = ppool.tile([128, BPG, D], bf16)
                        vb = vl[:, 0, :, None].broadcast_to([128, BPG, D])
                        nc.vector.tensor_tensor(out=xgb[:], in0=xg[:], in1=vb, op=mult)
                        oh = ppool.tile([128, BPG, 128], bf16)
                        lb = vl[:, 1, :, None].broadcast_to([128, BPG, 128])
                        ib = iota[:, None, :].broadcast_to([128, BPG, 128])
                        nc.vector.tensor_tensor(out=oh[:], in0=lb, in1=ib, op=iseq)
                        for b in range(BPG):
                            w, first, last = blkmap[j][g * BPG + b]
                            if first:
                                pst = pepool.tile([128, D], f32)
                            nc.tensor.matmul(
                                pst[:, :], lhsT=oh[:, b, :], rhs=xgb[:, b, :],
                                start=first, stop=last,
                            )
                            if last:
                                nc.vector.tensor_tensor(
                                    out=side_s[:, w, :], in0=side_s[:, w, :],
                                    in1=pst[:, :], op=add,
                                )
                        gpos += 1

                for t in range(NT):
                    r0 = t * 128
                    P = min(128, RPC - r0)
                    sz = dpool.tile([128, 2 * D], f32)
                    nc.scalar.copy(sz[:, 0:D], side_s[:, t, :])
                    nc.vector.tensor_mul(sz[:, D:2 * D], side_s[:, t, :], eown_s[:, t, :])
                    szT_p = pspool.tile([128, 128], f32)
                    nc.tensor.transpose(out=szT_p[:], in_=sz[:], identity=ident[:])
                    szT_s = dpool.tile([128, 128], f32)
                    nc.scalar.copy(szT_s[:], szT_p[:])
                    y_p = pspool.tile([128, D], f32)
                    nc.tensor.matmul(
                        y_p[:], lhsT=szT_s[:], rhs=wtile[:, k * D:(k + 1) * D],
                        start=True, stop=True,
                    )
                    yt = dpool.tile([128, D], f32)
                    nc.vector.tensor_add(yt[:], y_p[:], btile[:, k * D:(k + 1) * D])
                    nc.vector.scalar_tensor_tensor(
                        out=yt[:], in0=yt[:], scalar=0.2, in1=yt[:],
                        op0=mult, op1=amax,
                    )
                    nc.scalar.copy(eown_s[:, t, :], yt[:])
                    nc.sync.dma_start(mines[k][r0:r0 + P, :], yt[:P, :])

                nc.gpsimd.collective_compute(
                    "AllGather",
                    mybir.AluOpType.bypass,
                    replica_groups=[list(range(NC))],
                    ins=[mines[k][:, :]],
                    outs=[egofs[k][:, :]],
                )

            srcs = [ego0_full] + egofs
            accs = []
            for s_i in range(2):
                acc = fpool.tile([128, 8, 4 * D], f32, name=f"acc{s_i}")
                for ti in range(4):
                    src = srcs[ti]
                    for p in range(4):
                        jch = (p // 2) if s_i == 0 else (2 + p % 2)
                        fi = fpool.tile([128, CAP2 // 16], i16)
                        nc.sync.dma_start(fi[:], fidx[s_i, p, :, :])
                        g = fpool.tile([128, 2, D], f32)
                        nc.gpsimd.dma_gather(
                            g[:], src[jch * CHUNK:(jch + 1) * CHUNK, :], fi[:], CAP2, nreg_cap, D
                        )
                        dst = acc[:, 2 * p:2 * p + 2, ti * D:(ti + 1) * D]
                        if ti == 0:
                            nc.scalar.copy(dst, g[:])
                        else:
                            sq = fpool.tile([128, 2, D], f32)
                            nc.vector.tensor_mul(sq[:], g[:], g[:])
                            nr = fpool.tile([128, 2], f32)
                            nc.vector.reduce_sum(nr[:], sq[:], axis=mybir.AxisListType.X)
                            nc.scalar.activation(nr[:], nr[:], mybir.ActivationFunctionType.Sqrt)
                            nc.vector.tensor_scalar_max(nr[:], nr[:], EPS)
                            nc.vector.reciprocal(nr[:], nr[:])
                            nb = nr[:, :, None].broadcast_to([128, 2, D])
                            nc.vector.tensor_tensor(out=dst, in0=g[:], in1=nb, op=mult)
                accs.append(acc)
            u_acc, i_acc = accs
            prod = fpool.tile([128, 8, 4 * D], f32)
            nc.vector.tensor_mul(prod[:], u_acc[:], i_acc[:])
            pos_t = fpool.tile([128, 8], f32)
            nc.vector.reduce_sum(pos_t[:], prod[:], axis=mybir.AxisListType.X)
            nc.sync.dma_start(u_part[:, :].rearrange("(p f) d -> p f d", f=8), u_acc[:])
            nc.sync.dma_start(i_part[:, :].rearrange("(p f) d -> p f d", f=8), i_acc[:])
            nc.sync.dma_start(pos_part[:].rearrange("(p f) -> p f", f=8), pos_t[:])

    nc.finalize()
    return nc


def _run_timed(nc, in_maps, iters):
    # mirror of bass2jax.run_bass_via_pjrt multi-core branch, but keeps the
    # jitted executable so repeated launches can be wall-clock timed.
    import time

    import jax
    from jax.experimental.shard_map import shard_map
    from jax.sharding import Mesh, NamedSharding, PartitionSpec

    from concourse import bass2jax

    bass2jax.install_neuronx_cc_hook()
    partition_name = nc.partition_id_tensor.name if nc.partition_id_tensor else None
    in_names, out_names, out_avals, zero_outs = [], [], [], []
    for alloc in nc.m.functions[0].allocations:
        if not isinstance(alloc, mybir.MemoryLocationSet):
            continue
        name = alloc.memorylocations[0].name
        if alloc.kind == "ExternalInput":
            if name != partition_name:
                in_names.append(name)
        elif alloc.kind == "ExternalOutput":
            shape = tuple(alloc.tensor_shape)
            dtype = mybir.dt.np(alloc.dtype)
            out_names.append(name)
            out_avals.append(jax.core.ShapedArray(shape, dtype))
            zero_outs.append(np.zeros(shape, dtype))
    n_params = len(in_names)
    n_outs = len(out_avals)
    in_names.extend(out_names)
    if partition_name is not None:
        in_names.append(partition_name)
    donate = tuple(range(n_params, n_params + n_outs))

    def _body(*args):
        operands = list(args)
        if partition_name is not None:
            operands.append(bass2jax.partition_id_tensor())
        outs = bass2jax._bass_exec_p.bind(
            *operands,
            out_avals=tuple(out_avals),
            in_names=tuple(in_names),
            out_names=tuple(out_names),
            lowering_input_output_aliases=(),
            sim_require_finite=True,
            sim_require_nnan=True,
            nc=nc,
        )
        return tuple(outs)

    devices = jax.devices()[:NC]
    mesh = Mesh(np.asarray(devices), ("core",))
    in_specs = (PartitionSpec("core"),) * (n_params + n_outs)
    out_specs = (PartitionSpec("core"),) * len(out_names)
    sharded = jax.jit(
        shard_map(_body, mesh=mesh, in_specs=in_specs, out_specs=out_specs, check_rep=False),
        donate_argnums=donate,
        keep_unused=True,
    )
    per_core = [[np.asarray(m[name]) for name in in_names[:n_params]] for m in in_maps]
    concat_in = [
        np.concatenate([per_core[c][i] for c in range(NC)], axis=0)
        for i in range(n_params)
    ]
    concat_zeros = [
        np.zeros((NC * z.shape[0], *z.shape[1:]), z.dtype) for z in zero_outs
    ]
    sh = NamedSharding(mesh, PartitionSpec("core"))
    dev_in = [jax.device_put(a, sh) for a in concat_in]
    out_arrs = sharded(*dev_in, *[jax.device_put(z, sh) for z in concat_zeros])
    jax.block_until_ready(out_arrs)
    results = [
        {
            name: np.asarray(out_arrs[i]).reshape(NC, *out_avals[i].shape)[c]
            for i, name in enumerate(out_names)
        }
        for c in range(NC)
    ]
    times = []
    for _ in range(iters):
        zs = [jax.device_put(z, sh) for z in concat_zeros]
        jax.block_until_ready(zs)
        t0 = time.perf_counter()
        o = sharded(*dev_in, *zs)
        jax.block_until_ready(o)
        times.append(time.perf_counter() - t0)
    LAST["times"] = times
    return results, (min(times) if times else None)


def _wrap_idx(arr2d):
    # [nb, n] -> [nb, 128, n//16]; idx layout wrapped[p, s] = x[s*16 + p%16]
    nb, n = arr2d.shape
    w = arr2d.reshape(nb, n // 16, 16).transpose(0, 2, 1)
    return np.ascontiguousarray(np.tile(w, (1, 8, 1)))


def kernel(user, item_i, adj_row, adj_col, adj_val, user_emb, item_emb, W_gc, b_gc, W_bi, b_bi):
    user = np.asarray(user)
    item_i = np.asarray(item_i)
    adj_row = np.asarray(adj_row).astype(np.int64)
    adj_col = np.asarray(adj_col).astype(np.int64)
    adj_val = np.asarray(adj_val, dtype=np.float32)
    user_emb = np.asarray(user_emb, dtype=np.float32)
    item_emb = np.asarray(item_emb, dtype=np.float32)
    W_gc = np.asarray(W_gc, dtype=np.float32)
    b_gc = np.asarray(b_gc, dtype=np.float32)
    W_bi = np.asarray(W_bi, dtype=np.float32)
    b_bi = np.asarray(b_bi, dtype=np.float32)

    ego0 = np.ascontiguousarray(np.concatenate([user_emb, item_emb], axis=0))
    w_cat = np.ascontiguousarray(np.concatenate([W_gc, W_bi], axis=1))  # [L,128,64]
    b_sum = (b_gc[:, 0, :] + b_bi[:, 0, :])[:, None, :]                 # [L,1,64]
    b_rep = np.ascontiguousarray(np.broadcast_to(b_sum, (L, 128, D)), dtype=np.float32)
    iota_m = np.broadcast_to(np.arange(128, dtype=np.float32)[None, :], (128, 128))
    iota_m = np.ascontiguousarray(iota_m)

    # ---- edge partition: sort by (dest core, source chunk, dest window) ----
    core = adj_row // RPC
    r_loc = adj_row % RPC
    w_arr = r_loc // 128
    ld_arr = (r_loc % 128).astype(np.float32)
    chunk = adj_col // CHUNK
    col_loc = (adj_col % CHUNK).astype(np.int16)
    # sort by (group, source col) — col order inside a group is free and
    # gives the SWDGE gather engine locality in the source table
    key = ((core * NCH + chunk) * NT + w_arr) * CHUNK + (adj_col % CHUNK)
    order = np.argsort(key, kind="stable")
    col_s = col_loc[order]
    val_s = adj_val[order]
    ld_s = ld_arr[order]
    bounds = np.searchsorted(key[order], np.arange(NC * NCH * NT + 1) * CHUNK)
    cnt = np.diff(bounds).reshape(NC, NCH, NT)

    # blocks per (chunk, window): max over cores, so one program fits all
    NBW = (-(-cnt // 128)).max(axis=0)          # [NCH, NT]
    pad_j = (-NBW.sum(axis=1)) % BPG
    NBW[:, NT - 1] += pad_j                     # pad to whole gathers on last window
    NG_j = [int(NBW[j].sum()) // BPG for j in range(NCH)]
    NGT = sum(NG_j)
    offs = np.zeros((NCH, NT), np.int64)        # block offset of window w in chunk j
    for j in range(NCH):
        offs[j, 1:] = np.cumsum(NBW[j])[:-1]
    blkmap = []
    for j in range(NCH):
        m = []
        for w in range(NT):
            nb = int(NBW[j, w])
            for b in range(nb):
                m.append((w, b == 0, b == nb - 1))
        blkmap.append(m)

    gidx_all = np.empty((NC, NGT, 128, EB // 16), np.int16)
    vld_all = np.empty((NC, NGT, 128, 2, BPG), np.float32)
    for c in range(NC):
        gpos = 0
        for j in range(NCH):
            totb = int(NBW[j].sum())
            cl = np.zeros(totb * 128, np.int16)
            vv = np.zeros(totb * 128, np.float32)
            lv = np.zeros(totb * 128, np.float32)
            for w in range(NT):
                kk = (c * NCH + j) * NT + w
                s, e = int(bounds[kk]), int(bounds[kk + 1])
                o = int(offs[j, w]) * 128
                cl[o:o + e - s] = col_s[s:e]
                vv[o:o + e - s] = val_s[s:e]
                lv[o:o + e - s] = ld_s[s:e]
            ng = totb // BPG
            gidx_all[c, gpos:gpos + ng] = _wrap_idx(cl.reshape(ng, EB))
            vld_all[c, gpos:gpos + ng, :, 0, :] = vv.reshape(ng, BPG, 128).transpose(0, 2, 1)
            vld_all[c, gpos:gpos + ng, :, 1, :] = lv.reshape(ng, BPG, 128).transpose(0, 2, 1)
            gpos += ng

    # ---- final-phase batch grouping by (u-chunk, i-chunk) pair ----
    fidx_all = np.empty((NC, 2, 4, 128, CAP2 // 16), np.int16)
    slots_all = []
    for c in range(NC):
        u_ids = user[c * BPC:(c + 1) * BPC].astype(np.int64)
        i_ids = item_i[c * BPC:(c + 1) * BPC].astype(np.int64) + N_USER
        pair = (u_ids // CHUNK) * 2 + (i_ids // CHUNK - 2)
        slot_of = np.empty(BPC, np.int64)
        fx = np.zeros((2, 4, CAP2), np.int64)
        for p in range(4):
            pos = np.nonzero(pair == p)[0]
            pcnt = len(pos)
            assert pcnt <= CAP2, f"pair group overflow: core {c} pair {p} cnt {pcnt}"
            rank = np.arange(pcnt)
            slot_of[pos] = (rank % 128) * 8 + 2 * p + rank // 128
            fx[0, p, :pcnt] = u_ids[pos] - (p // 2) * CHUNK
            fx[1, p, :pcnt] = i_ids[pos] - (2 + p % 2) * CHUNK
        fw = fx.astype(np.int16).reshape(2, 4, CAP2 // 16, 16).transpose(0, 1, 3, 2)
        fidx_all[c] = np.tile(fw, (1, 1, 8, 1))
        slots_all.append(slot_of)

    nc = _build_program(NG_j, blkmap)

    ego0_pad = np.zeros((RPC_PAD, D), np.float32)
    in_maps = []
    for c in range(NC):
        ego0_pad_c = ego0_pad.copy()
        ego0_pad_c[:RPC] = ego0[c * RPC:(c + 1) * RPC]
        in_maps.append({
            "ego0_full": ego0,
            "ego0_own": ego0_pad_c,
            "w_cat": w_cat,
            "b_rep": b_rep,
            "iota_m": iota_m,
            "gidx": gidx_all[c],
            "vld": vld_all[c],
            "fidx": np.ascontiguousarray(fidx_all[c]),
        })

    trace = os.environ.get(TRACE_ENV, "0") == "1"
    time_iters = int(os.environ.get("NGCF_TIME_ITERS", "0"))
    if time_iters > 0 and not trace:
        res_list, t_best = _run_timed(nc, in_maps, time_iters)
        LAST["exec_time_ns"] = None if t_best is None else int(t_best * 1e9)
    else:
        res = run_bass_kernel_spmd(nc, in_maps, core_ids=list(range(NC)), trace=trace)
        LAST["exec_time_ns"] = res.exec_time_ns
        res_list = res.results

    u_g = np.empty((BATCH, 4 * D), np.float32)
    i_g = np.empty((BATCH, 4 * D), np.float32)
    pos = np.empty((BATCH,), np.float32)
    for c in range(NC):
        so = slots_all[c]
        rc = res_list[c]
        u_g[c * BPC:(c + 1) * BPC] = rc["u_part"][so]
        i_g[c * BPC:(c + 1) * BPC] = rc["i_part"][so]
        pos[c * BPC:(c + 1) * BPC] = rc["pos_part"][so]
    return (u_g, i_g, pos)
